# revision 25
# baseline (speedup 1.0000x reference)
"""Multi-head attention Trainium2 Bass kernel (v2, bf16).

Problem: B=2, S=2048, D=1024, H=16 heads, DH=64, causal (or arbitrary) mask.
Sharding: 8 cores = data-parallel over B (2) x tensor-parallel over head
groups (4 groups of 4 heads). Each core computes QKV projections for its
head group, attention for its 4 heads, and a partial output projection
(attended @ Wo-shard). Host sums the 4 partials per batch and adds bo.

v2 vs v1:
  - all matmul operands bf16 (f32 PSUM); exp output bf16; output bf16
    (host upcasts + reduces)
  - x is pre-transposed on the host -> no PE transposes, no DVE xT copies
  - program order interleaves qkv(sb+1) and proj(qb-1) with the ACT-bound
    attention(qb) so the PE always has filler work
  - PSUM budget: scores 2x[128,2,512] (4 banks) + accs 2x[65,512] (2) +
    qkv/proj 2x[128,512] (2) = 8 banks

Core-local design ("transposed scores" formulation):
  xT   [1024, S]    bf16, host-pretransposed, DMA'd per (d, sb) chunk
  qT,kT [256, S]    W.T @ xT  (2 tiles of [128, S] per, one per head-pair)
  v'   [S, 4, 65]   v natural + ones column per head (PV matmul emits the
                    softmax denominator as row 64)
  sT   [kv, q]      scores transposed = kT-chunk.T @ qT; K=64 matmuls
                    row-tiled (two heads on the two 64-row PE tile rows)
  expT exp(s/8)     bf16; causality: kv-blocks > q skipped, ranges
                    restricted, upper-tri 0/1 template on diagonal blocks
  attT [65, q]      v'.T @ expT accumulated over kv chunks; row 64 =
                    denom; normalize via reciprocal + partition_broadcast
                    + multiply -> attT [128, S] bf16
  out  [S, 1024]    attT.T @ Wo-shard bf16 (partial; host reduces)
"""
import numpy as np

B, S, D = 2, 2048, 1024
H, DH = 16, 64
NCORES = 8
HPC = 4              # heads per core
DIMS = HPC * DH      # 256 projection dims per core
NSB = S // 512       # 4 q/s blocks of 512
NST = S // 128       # 16 s tiles of 128
NDC = D // 128       # 8 contraction chunks

_PROG_CACHE = {}


def build_program(mode="causal", has_bias=False, reps=1, phases=None):
    import concourse.bacc as bacc
    import concourse.mybir as mybir
    import concourse.tile as tile

    DT = mybir.dt.float32
    BF = mybir.dt.bfloat16
    Act = mybir.ActivationFunctionType
    MUL = mybir.AluOpType.mult

    nc = bacc.Bacc("TRN2", target_bir_lowering=False, debug=False,
                   num_devices=NCORES)

    ndc = NDC + 1 if has_bias else NDC
    d_in = ndc * 128
    xt = nc.dram_tensor("xt", [d_in, S], BF, kind="ExternalInput")
    wq = nc.dram_tensor("wq", [d_in, DIMS], BF, kind="ExternalInput")
    wk = nc.dram_tensor("wk", [d_in, DIMS], BF, kind="ExternalInput")
    wv = nc.dram_tensor("wv", [d_in, DIMS], BF, kind="ExternalInput")
    wo = nc.dram_tensor("wo", [DIMS, D], BF, kind="ExternalInput")
    # consts: [:,0:4] ones | [:,4:388] zeros | [:,388:516] upper-tri ones
    # | [:,516:4612] ones (one-shot DMA into v_all's denominator columns)
    cin = nc.dram_tensor("cin", [128, 4612], BF, kind="ExternalInput")
    if mode == "general":
        mtin = nc.dram_tensor("maskt", [S, S], BF, kind="ExternalInput")
    outp = nc.dram_tensor("outp", [S, D], BF, kind="ExternalOutput")

    with tile.TileContext(nc) as tc:
        with (
            tc.tile_pool(name="pw", bufs=1) as pw,
            tc.tile_pool(name="pxa", bufs=1) as pxa,
            tc.tile_pool(name="pqk", bufs=1) as pqk,
            tc.tile_pool(name="pv", bufs=1) as pv,
            tc.tile_pool(name="patt", bufs=1) as patt,
            tc.tile_pool(name="pexp", bufs=8) as pexp,
            tc.tile_pool(name="pmask", bufs=4) as pmask,
            tc.tile_pool(name="pmisc", bufs=4) as pmisc,
            tc.tile_pool(name="pout", bufs=4) as pout,
            tc.tile_pool(name="psS", bufs=2, space="PSUM") as psS,
            tc.tile_pool(name="psA", bufs=2, space="PSUM") as psA,
            tc.tile_pool(name="psQ", bufs=2, space="PSUM") as psQ,
        ):
            def body():
                # zero the score-psum slots: exp reads full banks while the
                # causal matmuls write only [off:512), so first use would
                # otherwise read uninitialized PSUM (exp -> inf -> NaN)
                for _ in range(2):
                    pini = psS.tile([128, 2, 512], DT, tag="sc")
                    nc.vector.memset(pini[:], 0.0)
                # ---- weights + consts ----
                wqt = pw.tile([128, ndc, DIMS], BF, tag="wqt")
                wkt = pw.tile([128, ndc, DIMS], BF, tag="wkt")
                wvt = pw.tile([128, ndc, DIMS], BF, tag="wvt")
                wot = pw.tile([128, 2, D], BF, tag="wot")
                cst = pw.tile([128, 516], BF, tag="cst")
                # qk weights + x(0) first so the first matmuls start ASAP
                nc.sync.dma_start(wqt[:], wq.rearrange("(d p) j -> p d j", p=128))
                nc.sync.dma_start(wkt[:], wk.rearrange("(d p) j -> p d j", p=128))

                xall = pxa.tile([128, ndc, S], BF, tag="xall")
                qTs = [pqk.tile([128, S], BF, tag=f"qT{t}", name=f"qTs{t}")
                       for t in range(2)]
                kTs = [pqk.tile([128, S], BF, tag=f"kT{t}", name=f"kTs{t}")
                       for t in range(2)]
                # v' = [v | 64 ones cols]: the PV matmul then emits the
                # softmax denominator pre-replicated on partitions 64:128,
                # so no gpsimd partition_broadcast is needed.
                v_all = pv.tile([128, NST, HPC, 128], BF, tag="v")
                nc.sync.dma_start(
                    v_all[:, :, :, 64:128],
                    cin[:, 516:4612].rearrange("p (s h e) -> p s h e",
                                               s=NST, h=HPC))
                attTs = [patt.tile([128, S], BF, tag=f"aT{t}", name=f"attTs{t}")
                         for t in range(2)]

                xt_r = xt.rearrange("(d p) s -> p d s", p=128)

                def load_x(sb):
                    c0 = sb * 512
                    nc.sync.dma_start(xall[:, :, c0:c0 + 512],
                                      xt_r[:, :, c0:c0 + 512])

                def qkv(sb):
                    # two psum chains interleaved -> alternating-bank matmuls
                    c0 = sb * 512
                    for w3, dst in ((wqt, qTs), (wkt, kTs)):
                        pss = [psQ.tile([128, 512], DT, tag="mm",
                                        name=f"qk{t}") for t in range(2)]
                        for d in range(ndc):
                            for t in range(2):
                                nc.tensor.matmul(
                                    pss[t][:], w3[:, d, t * 128:(t + 1) * 128],
                                    xall[:, d, c0:c0 + 512],
                                    start=(d == 0), stop=(d == ndc - 1))
                        for t in range(2):
                            nc.vector.tensor_copy(dst[t][:, c0:c0 + 512],
                                                  pss[t][:])
                    for sp in range(2):
                        sts = (sb * 4 + sp * 2, sb * 4 + sp * 2 + 1)
                        pss = [psQ.tile([128, 256], DT, tag="mm",
                                        name=f"v{i}") for i in range(2)]
                        for d in range(ndc):
                            for i, st in enumerate(sts):
                                nc.tensor.matmul(
                                    pss[i][:],
                                    xall[:, d, st * 128:(st + 1) * 128],
                                    wvt[:, d, :],
                                    start=(d == 0), stop=(d == ndc - 1))
                        for i, st in enumerate(sts):
                            nc.vector.tensor_copy(
                                v_all[:, st, :, 0:64],
                                pss[i][:].rearrange("p (h e) -> p h e", h=HPC))

                def attention(qb, pair):
                    nkv = 4 * qb + 4 if mode == "causal" else NST
                    ncg = nkv // 2
                    q0 = qb * 512
                    accs = [psA.tile([128, 512], DT, tag="acc", name=f"acc{i}")
                            for i in range(2)]
                    LAG = 2
                    exp_q = {}
                    for u in range(ncg + LAG):
                        if u < ncg:
                            cg = u
                            if mode == "general":
                                mt = pmask.tile([128, 2, 512], BF, tag="mt")
                                for j in range(2):
                                    c = 2 * cg + j
                                    nc.sync.dma_start(
                                        mt[:, j, :],
                                        mtin[c * 128:(c + 1) * 128, q0:q0 + 512])
                            for h2 in range(2):
                                lo, hi = h2 * 64, (h2 + 1) * 64
                                scps = psS.tile([128, 2, 512], DT, tag="sc")
                                for j in range(2):
                                    c = 2 * cg + j
                                    off = (max(0, c * 128 - q0)
                                           if mode == "causal" else 0)
                                    nc.tensor.matmul(
                                        scps[:, j, off:512],
                                        kTs[pair][lo:hi, c * 128:(c + 1) * 128],
                                        qTs[pair][lo:hi, q0 + off:q0 + 512],
                                        start=True, stop=True)
                                expt = pexp.tile([128, 2, 512], BF, tag="exp")
                                nc.scalar.activation(expt[:], scps[:], Act.Exp,
                                                     scale=0.125)
                                # kill-multiply on DVE (bf16 2x mode); the
                                # gpsimd engine is far slower than modeled
                                eng = nc.vector
                                for j in range(2):
                                    c = 2 * cg + j
                                    if mode == "causal" and c * 128 >= q0:
                                        off = c * 128 - q0
                                        eng.tensor_tensor(
                                            expt[:, j, 0:off + 128],
                                            expt[:, j, 0:off + 128],
                                            cst[:, 388 - off:516], MUL)
                                    elif mode == "general":
                                        eng.tensor_tensor(
                                            expt[:, j, :], expt[:, j, :],
                                            mt[:, j, :], MUL)
                                exp_q[(cg, h2)] = expt
                        if u >= LAG:
                            cg = u - LAG
                            es = [exp_q.pop((cg, 0)), exp_q.pop((cg, 1))]
                            # j-outer so consecutive matmuls alternate the
                            # two accumulator banks
                            for j in range(2):
                                c = 2 * cg + j
                                for h2 in range(2):
                                    nc.tensor.matmul(
                                        accs[h2][:],
                                        v_all[:, c, pair * 2 + h2, :],
                                        es[h2][:, j, :],
                                        start=(c == 0), stop=(c == nkv - 1))
                    for h2 in range(2):
                        # accs rows 64:128 hold the denominator (replicated
                        # by the ones columns of v'); normalize on DVE only
                        if phases == "nonorm":
                            nc.vector.tensor_copy(
                                attTs[pair][h2 * 64:(h2 + 1) * 64,
                                            q0:q0 + 512],
                                accs[h2][0:64, :])
                            continue
                        if phases == "denom":
                            nc.vector.tensor_copy(
                                attTs[pair][h2 * 64:(h2 + 1) * 64,
                                            q0:q0 + 512],
                                accs[h2][64:128, :])
                            continue
                        recipb = pmisc.tile([64, 512], DT, tag="recipb")
                        nc.vector.reciprocal(recipb[:], accs[h2][64:128, :])
                        nc.vector.tensor_tensor(
                            attTs[pair][h2 * 64:(h2 + 1) * 64, q0:q0 + 512],
                            accs[h2][0:64, :], recipb[:], MUL)

                def proj(qb):
                    # t-outer so the attT stationary is loaded once per two
                    # matmuls (two psum chains, one per output half)
                    for stl in range(4):
                        st = qb * 4 + stl
                        ot = pout.tile([128, D], BF, tag="out")
                        pss = [psQ.tile([128, 512], DT, tag="mm",
                                        name=f"pj{half}") for half in range(2)]
                        for t in range(2):
                            for half in range(2):
                                nc.tensor.matmul(
                                    pss[half][:],
                                    attTs[t][:, st * 128:(st + 1) * 128],
                                    wot[:, t, half * 512:(half + 1) * 512],
                                    start=(t == 0), stop=(t == 1))
                        for half in range(2):
                            nc.vector.tensor_copy(
                                ot[:, half * 512:(half + 1) * 512],
                                pss[half][:])
                        nc.sync.dma_start(outp[st * 128:(st + 1) * 128, :],
                                          ot[:])

                load_x(0)
                nc.sync.dma_start(wvt[:], wv.rearrange("(d p) j -> p d j", p=128))
                nc.sync.dma_start(wot[:], wo.rearrange("(t p) j -> p t j", p=128))
                nc.sync.dma_start(cst[:], cin[:, 0:516])
                if mode == "causal":
                    # att(qb) only reads kv chunks <= qb, so qkv(sb+1) can
                    # interleave with (and fill PE gaps of) att(sb)
                    qkv(0)
                    load_x(1); qkv(1)
                    attention(0, 0); attention(0, 1)
                    load_x(2); qkv(2)
                    attention(1, 0); attention(1, 1)
                    load_x(3); qkv(3)
                    proj(0)
                    attention(2, 0); attention(2, 1)
                    proj(1)
                    attention(3, 0)
                    proj(2)
                    attention(3, 1)
                    proj(3)
                else:
                    # full attention reads every kv chunk: all qkv first
                    qkv(0)
                    load_x(1); qkv(1)
                    load_x(2); qkv(2)
                    load_x(3); qkv(3)
                    for qb in range(NSB):
                        attention(qb, 0); attention(qb, 1)
                        proj(qb)

            if reps == 1:
                body()
            else:
                with tc.For_i(0, reps, 1):
                    body()

    nc.compile()
    return nc


def _consts_array():
    c = np.zeros((128, 4612), dtype=np.float32)
    c[:, 0:4] = 1.0
    c[:, 388:516] = np.triu(np.ones((128, 128), np.float32))
    c[:, 516:4612] = 1.0
    return c


def _bf16(a):
    import ml_dtypes
    return np.ascontiguousarray(np.asarray(a, np.float32)).astype(
        ml_dtypes.bfloat16)


def make_in_maps(x, mask, Wq, bq, Wk, bk, Wv, bv, Wo, bo):
    x = np.asarray(x, np.float32)
    m = np.asarray(mask)[0, 0]
    mb = (m != 0)
    if mb.all():
        mode = "none"
    elif np.array_equal(mb, np.tril(np.ones((S, S), bool))):
        mode = "causal"
    else:
        mode = "general"
    has_bias = bool(np.any(bq) or np.any(bk) or np.any(bv))

    Wq = np.asarray(Wq, np.float32)
    Wk = np.asarray(Wk, np.float32)
    Wv = np.asarray(Wv, np.float32)
    Wo = np.asarray(Wo, np.float32)
    if has_bias:
        # contraction dim padded to 9*128: row D = bias (ones row in xT)
        pad = np.zeros((128, H * DH), np.float32)
        Wq = np.concatenate([Wq, pad], 0)
        Wk = np.concatenate([Wk, pad], 0)
        Wv = np.concatenate([Wv, pad], 0)
        Wq[D] = np.asarray(bq, np.float32)
        Wk[D] = np.asarray(bk, np.float32)
        Wv[D] = np.asarray(bv, np.float32)
    consts = _consts_array()
    maskt = mb.T.astype(np.float32) if mode == "general" else None

    in_maps = []
    for c in range(NCORES):
        b, hg = divmod(c, HPC)
        cols = slice(hg * DIMS, (hg + 1) * DIMS)
        xtb = x[b].T
        if has_bias:
            xtb = np.concatenate(
                [xtb, np.ones((1, S), np.float32),
                 np.zeros((127, S), np.float32)], 0)
        im = {
            "xt": _bf16(xtb),
            "wq": _bf16(Wq[:, cols]),
            "wk": _bf16(Wk[:, cols]),
            "wv": _bf16(Wv[:, cols]),
            "wo": _bf16(Wo[hg * DIMS:(hg + 1) * DIMS, :]),
            "cin": _bf16(consts),
        }
        if maskt is not None:
            im["maskt"] = _bf16(maskt)
        in_maps.append(im)
    return in_maps, mode, has_bias


def gather_output(results, bo):
    out = np.zeros((B, S, D), dtype=np.float32)
    for c in range(NCORES):
        out[c // HPC] += np.asarray(results[c]["outp"], dtype=np.float32)
    out += np.asarray(bo, np.float32)[None, None, :]
    return out


def get_program(mode, has_bias, reps=1, phases=None):
    key = (mode, has_bias, reps, phases)
    if key not in _PROG_CACHE:
        _PROG_CACHE[key] = build_program(mode, has_bias, reps, phases)
    return _PROG_CACHE[key]


def run(in_maps, mode, has_bias, reps=1, phases=None, **kwargs):
    from concourse.bass_utils import run_bass_kernel_spmd
    nc = get_program(mode, has_bias, reps, phases)
    return run_bass_kernel_spmd(nc, in_maps, core_ids=list(range(NCORES)),
                                **kwargs)


def kernel(x, mask, Wq, bq, Wk, bk, Wv, bv, Wo, bo):
    in_maps, mode, has_bias = make_in_maps(x, mask, Wq, bq, Wk, bk, Wv, bv,
                                           Wo, bo)
    r = run(in_maps, mode, has_bias, reps=1)
    return gather_output(r.results, bo)


# revision 32
# speedup vs baseline: 1.0916x; 1.0916x over previous
"""Multi-head attention Trainium2 Bass kernel (v2, bf16).

Problem: B=2, S=2048, D=1024, H=16 heads, DH=64, causal (or arbitrary) mask.
Sharding: 8 cores = data-parallel over B (2) x tensor-parallel over head
groups (4 groups of 4 heads). Each core computes QKV projections for its
head group, attention for its 4 heads, and a partial output projection
(attended @ Wo-shard). Host sums the 4 partials per batch and adds bo.

v2 vs v1:
  - all matmul operands bf16 (f32 PSUM); exp output bf16; output bf16
    (host upcasts + reduces)
  - x is pre-transposed on the host -> no PE transposes, no DVE xT copies
  - program order interleaves qkv(sb+1) and proj(qb-1) with the ACT-bound
    attention(qb) so the PE always has filler work
  - PSUM budget: scores 2x[128,2,512] (4 banks) + accs 2x[65,512] (2) +
    qkv/proj 2x[128,512] (2) = 8 banks

Core-local design ("transposed scores" formulation):
  xT   [1024, S]    bf16, host-pretransposed, DMA'd per (d, sb) chunk
  qT,kT [256, S]    W.T @ xT  (2 tiles of [128, S] per, one per head-pair)
  v'   [S, 4, 65]   v natural + ones column per head (PV matmul emits the
                    softmax denominator as row 64)
  sT   [kv, q]      scores transposed = kT-chunk.T @ qT; K=64 matmuls
                    row-tiled (two heads on the two 64-row PE tile rows)
  expT exp(s/8)     bf16; causality: kv-blocks > q skipped, ranges
                    restricted, upper-tri 0/1 template on diagonal blocks
  attT [65, q]      v'.T @ expT accumulated over kv chunks; row 64 =
                    denom; normalize via reciprocal + partition_broadcast
                    + multiply -> attT [128, S] bf16
  out  [S, 1024]    attT.T @ Wo-shard bf16 (partial; host reduces)
"""
import numpy as np

B, S, D = 2, 2048, 1024
H, DH = 16, 64
NCORES = 8
HPC = 4              # heads per core
DIMS = HPC * DH      # 256 projection dims per core
NSB = S // 512       # 4 q/s blocks of 512
NST = S // 128       # 16 s tiles of 128
NDC = D // 128       # 8 contraction chunks

_PROG_CACHE = {}


def build_program(mode="causal", has_bias=False, reps=1, phases=None):
    import concourse.bacc as bacc
    import concourse.mybir as mybir
    import concourse.tile as tile

    DT = mybir.dt.float32
    BF = mybir.dt.bfloat16
    Act = mybir.ActivationFunctionType
    MUL = mybir.AluOpType.mult

    nc = bacc.Bacc("TRN2", target_bir_lowering=False, debug=False,
                   num_devices=NCORES)

    ndc = NDC + 1 if has_bias else NDC
    d_in = ndc * 128
    xt = nc.dram_tensor("xt", [d_in, S], BF, kind="ExternalInput")
    wq = nc.dram_tensor("wq", [d_in, DIMS], BF, kind="ExternalInput")
    wk = nc.dram_tensor("wk", [d_in, DIMS], BF, kind="ExternalInput")
    wv = nc.dram_tensor("wv", [d_in, DIMS], BF, kind="ExternalInput")
    wo = nc.dram_tensor("wo", [DIMS, D], BF, kind="ExternalInput")
    # consts: [:,0:4] ones | [:,4:388] zeros | [:,388:516] upper-tri ones
    # | [:,516:4612] ones (one-shot DMA into v_all's denominator columns)
    cin = nc.dram_tensor("cin", [128, 4612], BF, kind="ExternalInput")
    if mode == "general":
        mtin = nc.dram_tensor("maskt", [S, S], BF, kind="ExternalInput")
    outp = nc.dram_tensor("outp", [S, D], BF, kind="ExternalOutput")

    with tile.TileContext(nc) as tc:
        with (
            tc.tile_pool(name="pw", bufs=1) as pw,
            tc.tile_pool(name="pxa", bufs=1) as pxa,
            tc.tile_pool(name="pqk", bufs=1) as pqk,
            tc.tile_pool(name="pv", bufs=1) as pv,
            tc.tile_pool(name="patt", bufs=1) as patt,
            tc.tile_pool(name="pexp", bufs=8) as pexp,
            tc.tile_pool(name="pmask", bufs=4) as pmask,
            tc.tile_pool(name="pmisc", bufs=4) as pmisc,
            tc.tile_pool(name="pout", bufs=4) as pout,
            tc.tile_pool(name="psS", bufs=2, space="PSUM") as psS,
            tc.tile_pool(name="psA", bufs=2, space="PSUM") as psA,
            tc.tile_pool(name="psQ", bufs=2, space="PSUM") as psQ,
        ):
            def body():
                # zero the score-psum slots: exp reads full banks while the
                # causal matmuls write only [off:512), so first use would
                # otherwise read uninitialized PSUM (exp -> inf -> NaN)
                for _ in range(2):
                    pini = psS.tile([128, 2, 512], DT, tag="sc")
                    nc.vector.memset(pini[:], 0.0)
                # ---- weights + consts ----
                wqt = pw.tile([128, ndc, DIMS], BF, tag="wqt")
                wkt = pw.tile([128, ndc, DIMS], BF, tag="wkt")
                wvt = pw.tile([128, ndc, DIMS], BF, tag="wvt")
                wot = pw.tile([128, 2, D], BF, tag="wot")
                cst = pw.tile([128, 516], BF, tag="cst")
                # qk weights + x(0) first so the first matmuls start ASAP
                nc.sync.dma_start(wqt[:], wq.rearrange("(d p) j -> p d j", p=128))
                nc.sync.dma_start(wkt[:], wk.rearrange("(d p) j -> p d j", p=128))

                xall = pxa.tile([128, ndc, S], BF, tag="xall")
                qTs = [pqk.tile([128, S], BF, tag=f"qT{t}", name=f"qTs{t}")
                       for t in range(2)]
                kTs = [pqk.tile([128, S], BF, tag=f"kT{t}", name=f"kTs{t}")
                       for t in range(2)]
                # v' = [v | 64 ones cols]: the PV matmul then emits the
                # softmax denominator pre-replicated on partitions 64:128,
                # so no gpsimd partition_broadcast is needed.
                v_all = pv.tile([128, NST, HPC, 128], BF, tag="v")
                nc.sync.dma_start(
                    v_all[:, :, :, 64:128],
                    cin[:, 516:4612].rearrange("p (s h e) -> p s h e",
                                               s=NST, h=HPC))
                attTs = [patt.tile([128, S], BF, tag=f"aT{t}", name=f"attTs{t}")
                         for t in range(2)]

                xt_r = xt.rearrange("(d p) s -> p d s", p=128)

                def load_x(sb):
                    c0 = sb * 512
                    nc.sync.dma_start(xall[:, :, c0:c0 + 512],
                                      xt_r[:, :, c0:c0 + 512])

                peonly = phases == "peonly"

                def qkv(sb):
                    # two psum chains interleaved -> alternating-bank matmuls
                    c0 = sb * 512
                    for w3, dst in ((wqt, qTs), (wkt, kTs)):
                        pss = [psQ.tile([128, 512], DT, tag="mm",
                                        name=f"qk{t}") for t in range(2)]
                        for d in range(ndc):
                            for t in range(2):
                                nc.tensor.matmul(
                                    pss[t][:], w3[:, d, t * 128:(t + 1) * 128],
                                    xall[:, d, c0:c0 + 512],
                                    start=(d == 0), stop=(d == ndc - 1))
                        if peonly:
                            continue
                        for t in range(2):
                            nc.vector.tensor_copy(dst[t][:, c0:c0 + 512],
                                                  pss[t][:])
                    for sp in range(2):
                        sts = (sb * 4 + sp * 2, sb * 4 + sp * 2 + 1)
                        pss = [psQ.tile([128, 256], DT, tag="mm",
                                        name=f"v{i}") for i in range(2)]
                        for d in range(ndc):
                            for i, st in enumerate(sts):
                                nc.tensor.matmul(
                                    pss[i][:],
                                    xall[:, d, st * 128:(st + 1) * 128],
                                    wvt[:, d, :],
                                    start=(d == 0), stop=(d == ndc - 1))
                        if peonly:
                            continue
                        for i, st in enumerate(sts):
                            nc.vector.tensor_copy(
                                v_all[:, st, :, 0:64],
                                pss[i][:].rearrange("p (h e) -> p h e", h=HPC))

                def attention(qb, pair):
                    nkv = 4 * qb + 4 if mode == "causal" else NST
                    ncg = nkv // 2
                    q0 = qb * 512
                    accs = [psA.tile([128, 512], DT, tag="acc", name=f"acc{i}")
                            for i in range(2)]
                    if peonly:
                        # matmul stream only: PV reads a const tile, no
                        # exp/kill/normalize -> measures the PE serial floor
                        for cg in range(ncg):
                            for h2 in range(2):
                                scps = psS.tile([128, 2, 512], DT, tag="sc")
                                for j in range(2):
                                    c = 2 * cg + j
                                    off = (max(0, c * 128 - q0)
                                           if mode == "causal" else 0)
                                    nc.tensor.matmul(
                                        scps[:, j, off:512],
                                        kTs[pair][h2 * 64:(h2 + 1) * 64,
                                                  c * 128:(c + 1) * 128],
                                        qTs[pair][h2 * 64:(h2 + 1) * 64,
                                                  q0 + off:q0 + 512],
                                        start=True, stop=True)
                            for j in range(2):
                                c = 2 * cg + j
                                for h2 in range(2):
                                    nc.tensor.matmul(
                                        accs[h2][:],
                                        v_all[:, c, pair * 2 + h2, :],
                                        cst[:, 4:516],
                                        start=(c == 0), stop=(c == nkv - 1))
                        return
                    LAG = 2
                    exp_q = {}
                    for u in range(ncg + LAG):
                        if u < ncg:
                            cg = u
                            if mode == "general":
                                mt = pmask.tile([128, 2, 512], BF, tag="mt")
                                for j in range(2):
                                    c = 2 * cg + j
                                    nc.sync.dma_start(
                                        mt[:, j, :],
                                        mtin[c * 128:(c + 1) * 128, q0:q0 + 512])
                            for h2 in range(2):
                                lo, hi = h2 * 64, (h2 + 1) * 64
                                scps = psS.tile([128, 2, 512], DT, tag="sc")
                                for j in range(2):
                                    c = 2 * cg + j
                                    off = (max(0, c * 128 - q0)
                                           if mode == "causal" else 0)
                                    nc.tensor.matmul(
                                        scps[:, j, off:512],
                                        kTs[pair][lo:hi, c * 128:(c + 1) * 128],
                                        qTs[pair][lo:hi, q0 + off:q0 + 512],
                                        start=True, stop=True)
                                expt = pexp.tile([128, 2, 512], BF, tag="exp")
                                nc.scalar.activation(expt[:], scps[:], Act.Exp,
                                                     scale=0.125)
                                # kill-multiply on DVE (bf16 2x mode); the
                                # gpsimd engine is far slower than modeled
                                eng = nc.vector
                                for j in range(2):
                                    c = 2 * cg + j
                                    if mode == "causal" and c * 128 >= q0:
                                        off = c * 128 - q0
                                        eng.tensor_tensor(
                                            expt[:, j, 0:off + 128],
                                            expt[:, j, 0:off + 128],
                                            cst[:, 388 - off:516], MUL)
                                    elif mode == "general":
                                        eng.tensor_tensor(
                                            expt[:, j, :], expt[:, j, :],
                                            mt[:, j, :], MUL)
                                exp_q[(cg, h2)] = expt
                        if u >= LAG:
                            cg = u - LAG
                            es = [exp_q.pop((cg, 0)), exp_q.pop((cg, 1))]
                            # j-outer so consecutive matmuls alternate the
                            # two accumulator banks
                            for j in range(2):
                                c = 2 * cg + j
                                for h2 in range(2):
                                    nc.tensor.matmul(
                                        accs[h2][:],
                                        v_all[:, c, pair * 2 + h2, :],
                                        es[h2][:, j, :],
                                        start=(c == 0), stop=(c == nkv - 1))
                    for h2 in range(2):
                        # accs rows 64:128 hold the denominator (replicated
                        # by the ones columns of v'); normalize on DVE only
                        if phases == "nonorm":
                            nc.vector.tensor_copy(
                                attTs[pair][h2 * 64:(h2 + 1) * 64,
                                            q0:q0 + 512],
                                accs[h2][0:64, :])
                            continue
                        if phases == "denom":
                            nc.vector.tensor_copy(
                                attTs[pair][h2 * 64:(h2 + 1) * 64,
                                            q0:q0 + 512],
                                accs[h2][64:128, :])
                            continue
                        recipb = pmisc.tile([64, 512], DT, tag="recipb")
                        nc.vector.reciprocal(recipb[:], accs[h2][64:128, :])
                        nc.vector.tensor_tensor(
                            attTs[pair][h2 * 64:(h2 + 1) * 64, q0:q0 + 512],
                            accs[h2][0:64, :], recipb[:], MUL)

                def proj(qb):
                    # t-outer so the attT stationary is loaded once per two
                    # matmuls (two psum chains, one per output half)
                    for stl in range(4):
                        st = qb * 4 + stl
                        ot = pout.tile([128, D], BF, tag="out")
                        pss = [psQ.tile([128, 512], DT, tag="mm",
                                        name=f"pj{half}") for half in range(2)]
                        for t in range(2):
                            for half in range(2):
                                nc.tensor.matmul(
                                    pss[half][:],
                                    attTs[t][:, st * 128:(st + 1) * 128],
                                    wot[:, t, half * 512:(half + 1) * 512],
                                    start=(t == 0), stop=(t == 1))
                        if peonly:
                            continue
                        for half in range(2):
                            nc.vector.tensor_copy(
                                ot[:, half * 512:(half + 1) * 512],
                                pss[half][:])
                        nc.sync.dma_start(outp[st * 128:(st + 1) * 128, :],
                                          ot[:])

                load_x(0)
                nc.sync.dma_start(wvt[:], wv.rearrange("(d p) j -> p d j", p=128))
                nc.sync.dma_start(wot[:], wo.rearrange("(t p) j -> p t j", p=128))
                nc.sync.dma_start(cst[:], cin[:, 0:516])
                if mode == "causal":
                    # att(qb) only reads kv chunks <= qb: issue it as early
                    # as its deps allow so the ACT exp stream starts ~10us
                    # in; qkv(sb+1)/proj become the PE filler that the
                    # scheduler pulls forward whenever attention stalls on
                    # exp results.
                    qkv(0)
                    load_x(1)
                    attention(0, 0); attention(0, 1)
                    qkv(1)
                    load_x(2)
                    attention(1, 0); attention(1, 1)
                    qkv(2)
                    load_x(3)
                    attention(2, 0); attention(2, 1)
                    qkv(3)
                    attention(3, 0)
                    proj(0); proj(1)
                    attention(3, 1)
                    proj(2); proj(3)
                else:
                    # full attention reads every kv chunk: all qkv first
                    qkv(0)
                    load_x(1); qkv(1)
                    load_x(2); qkv(2)
                    load_x(3); qkv(3)
                    for qb in range(NSB):
                        attention(qb, 0); attention(qb, 1)
                        proj(qb)

            if reps == 1:
                body()
            else:
                with tc.For_i(0, reps, 1):
                    body()

    nc.compile()
    return nc


def _consts_array():
    c = np.zeros((128, 4612), dtype=np.float32)
    c[:, 0:4] = 1.0
    c[:, 388:516] = np.triu(np.ones((128, 128), np.float32))
    c[:, 516:4612] = 1.0
    return c


def _bf16(a):
    import ml_dtypes
    return np.ascontiguousarray(np.asarray(a, np.float32)).astype(
        ml_dtypes.bfloat16)


def make_in_maps(x, mask, Wq, bq, Wk, bk, Wv, bv, Wo, bo):
    x = np.asarray(x, np.float32)
    m = np.asarray(mask)[0, 0]
    mb = (m != 0)
    if mb.all():
        mode = "none"
    elif np.array_equal(mb, np.tril(np.ones((S, S), bool))):
        mode = "causal"
    else:
        mode = "general"
    has_bias = bool(np.any(bq) or np.any(bk) or np.any(bv))

    Wq = np.asarray(Wq, np.float32)
    Wk = np.asarray(Wk, np.float32)
    Wv = np.asarray(Wv, np.float32)
    Wo = np.asarray(Wo, np.float32)
    if has_bias:
        # contraction dim padded to 9*128: row D = bias (ones row in xT)
        pad = np.zeros((128, H * DH), np.float32)
        Wq = np.concatenate([Wq, pad], 0)
        Wk = np.concatenate([Wk, pad], 0)
        Wv = np.concatenate([Wv, pad], 0)
        Wq[D] = np.asarray(bq, np.float32)
        Wk[D] = np.asarray(bk, np.float32)
        Wv[D] = np.asarray(bv, np.float32)
    consts = _consts_array()
    maskt = mb.T.astype(np.float32) if mode == "general" else None

    in_maps = []
    for c in range(NCORES):
        b, hg = divmod(c, HPC)
        cols = slice(hg * DIMS, (hg + 1) * DIMS)
        xtb = x[b].T
        if has_bias:
            xtb = np.concatenate(
                [xtb, np.ones((1, S), np.float32),
                 np.zeros((127, S), np.float32)], 0)
        im = {
            "xt": _bf16(xtb),
            "wq": _bf16(Wq[:, cols]),
            "wk": _bf16(Wk[:, cols]),
            "wv": _bf16(Wv[:, cols]),
            "wo": _bf16(Wo[hg * DIMS:(hg + 1) * DIMS, :]),
            "cin": _bf16(consts),
        }
        if maskt is not None:
            im["maskt"] = _bf16(maskt)
        in_maps.append(im)
    return in_maps, mode, has_bias


def gather_output(results, bo):
    out = np.zeros((B, S, D), dtype=np.float32)
    for c in range(NCORES):
        out[c // HPC] += np.asarray(results[c]["outp"], dtype=np.float32)
    out += np.asarray(bo, np.float32)[None, None, :]
    return out


def get_program(mode, has_bias, reps=1, phases=None):
    key = (mode, has_bias, reps, phases)
    if key not in _PROG_CACHE:
        _PROG_CACHE[key] = build_program(mode, has_bias, reps, phases)
    return _PROG_CACHE[key]


def run(in_maps, mode, has_bias, reps=1, phases=None, **kwargs):
    from concourse.bass_utils import run_bass_kernel_spmd
    nc = get_program(mode, has_bias, reps, phases)
    return run_bass_kernel_spmd(nc, in_maps, core_ids=list(range(NCORES)),
                                **kwargs)


def kernel(x, mask, Wq, bq, Wk, bk, Wv, bv, Wo, bo):
    in_maps, mode, has_bias = make_in_maps(x, mask, Wq, bq, Wk, bk, Wv, bv,
                                           Wo, bo)
    r = run(in_maps, mode, has_bias, reps=1)
    return gather_output(r.results, bo)


# revision 40
# speedup vs baseline: 1.1519x; 1.0552x over previous
"""Multi-head attention Trainium2 Bass kernel (v2, bf16).

Problem: B=2, S=2048, D=1024, H=16 heads, DH=64, causal (or arbitrary) mask.
Sharding: 8 cores = data-parallel over B (2) x tensor-parallel over head
groups (4 groups of 4 heads). Each core computes QKV projections for its
head group, attention for its 4 heads, and a partial output projection
(attended @ Wo-shard). Host sums the 4 partials per batch and adds bo.

v2 vs v1:
  - all matmul operands bf16 (f32 PSUM); exp output bf16; output bf16
    (host upcasts + reduces)
  - x is pre-transposed on the host -> no PE transposes, no DVE xT copies
  - program order interleaves qkv(sb+1) and proj(qb-1) with the ACT-bound
    attention(qb) so the PE always has filler work
  - PSUM budget: scores 2x[128,2,512] (4 banks) + accs 2x[65,512] (2) +
    qkv/proj 2x[128,512] (2) = 8 banks

Core-local design ("transposed scores" formulation):
  xT   [1024, S]    bf16, host-pretransposed, DMA'd per (d, sb) chunk
  qT,kT [256, S]    W.T @ xT  (2 tiles of [128, S] per, one per head-pair)
  v'   [S, 4, 65]   v natural + ones column per head (PV matmul emits the
                    softmax denominator as row 64)
  sT   [kv, q]      scores transposed = kT-chunk.T @ qT; K=64 matmuls
                    row-tiled (two heads on the two 64-row PE tile rows)
  expT exp(s/8)     bf16; causality: kv-blocks > q skipped, ranges
                    restricted, upper-tri 0/1 template on diagonal blocks
  attT [65, q]      v'.T @ expT accumulated over kv chunks; row 64 =
                    denom; normalize via reciprocal + partition_broadcast
                    + multiply -> attT [128, S] bf16
  out  [S, 1024]    attT.T @ Wo-shard bf16 (partial; host reduces)
"""
import numpy as np

B, S, D = 2, 2048, 1024
H, DH = 16, 64
NCORES = 8
HPC = 4              # heads per core
DIMS = HPC * DH      # 256 projection dims per core
NSB = S // 512       # 4 q/s blocks of 512
NST = S // 128       # 16 s tiles of 128
NDC = D // 128       # 8 contraction chunks

_PROG_CACHE = {}


def build_program(mode="causal", has_bias=False, reps=1, phases=None):
    import concourse.bacc as bacc
    import concourse.mybir as mybir
    import concourse.tile as tile

    DT = mybir.dt.float32
    BF = mybir.dt.bfloat16
    Act = mybir.ActivationFunctionType
    MUL = mybir.AluOpType.mult

    nc = bacc.Bacc("TRN2", target_bir_lowering=False, debug=False,
                   num_devices=NCORES)

    ndc = NDC + 1 if has_bias else NDC
    d_in = ndc * 128
    xt = nc.dram_tensor("xt", [d_in, S], BF, kind="ExternalInput")
    wq = nc.dram_tensor("wq", [d_in, DIMS], BF, kind="ExternalInput")
    wk = nc.dram_tensor("wk", [d_in, DIMS], BF, kind="ExternalInput")
    wv = nc.dram_tensor("wv", [d_in, DIMS], BF, kind="ExternalInput")
    wo = nc.dram_tensor("wo", [DIMS, D], BF, kind="ExternalInput")
    # consts: [:,0:4] ones | [:,4:388] zeros | [:,388:516] upper-tri ones
    # | [:,516:4612] ones (one-shot DMA into v_all's denominator columns)
    cin = nc.dram_tensor("cin", [128, 4612], BF, kind="ExternalInput")
    if mode == "general":
        mtin = nc.dram_tensor("maskt", [S, S], BF, kind="ExternalInput")
    outp = nc.dram_tensor("outp", [S, D], BF, kind="ExternalOutput")

    with tile.TileContext(nc) as tc:
        with (
            tc.tile_pool(name="pw", bufs=1) as pw,
            tc.tile_pool(name="pxa", bufs=1) as pxa,
            tc.tile_pool(name="pqk", bufs=1) as pqk,
            tc.tile_pool(name="pv", bufs=1) as pv,
            tc.tile_pool(name="patt", bufs=1) as patt,
            tc.tile_pool(name="pexp", bufs=10) as pexp,
            tc.tile_pool(name="pmask", bufs=4) as pmask,
            tc.tile_pool(name="pmisc", bufs=4) as pmisc,
            tc.tile_pool(name="pout", bufs=4) as pout,
            tc.tile_pool(name="psS", bufs=2, space="PSUM") as psS,
            tc.tile_pool(name="psA", bufs=2, space="PSUM") as psA,
            tc.tile_pool(name="psQ", bufs=2, space="PSUM") as psQ,
        ):
            def body():
                # zero the score-psum slots: exp reads full banks while the
                # causal matmuls write only [off:512), so first use would
                # otherwise read uninitialized PSUM (exp -> inf -> NaN)
                for _ in range(2):
                    pini = psS.tile([128, 2, 512], DT, tag="sc")
                    nc.vector.memset(pini[:], 0.0)
                # ---- weights + consts ----
                wqt = pw.tile([128, ndc, DIMS], BF, tag="wqt")
                wkt = pw.tile([128, ndc, DIMS], BF, tag="wkt")
                wvt = pw.tile([128, ndc, DIMS], BF, tag="wvt")
                wot = pw.tile([128, 2, D], BF, tag="wot")
                cst = pw.tile([128, 516], BF, tag="cst")
                # qk weights + x(0) first so the first matmuls start ASAP
                nc.sync.dma_start(wqt[:], wq.rearrange("(d p) j -> p d j", p=128))
                nc.sync.dma_start(wkt[:], wk.rearrange("(d p) j -> p d j", p=128))

                xall = pxa.tile([128, ndc, S], BF, tag="xall")
                qTs = [pqk.tile([128, S], BF, tag=f"qT{t}", name=f"qTs{t}")
                       for t in range(2)]
                kTs = [pqk.tile([128, S], BF, tag=f"kT{t}", name=f"kTs{t}")
                       for t in range(2)]
                # v' = [v | 64 ones cols]: the PV matmul then emits the
                # softmax denominator pre-replicated on partitions 64:128,
                # so no gpsimd partition_broadcast is needed.
                v_all = pv.tile([128, NST, HPC, 128], BF, tag="v")
                nc.sync.dma_start(
                    v_all[:, :, :, 64:128],
                    cin[:, 516:4612].rearrange("p (s h e) -> p s h e",
                                               s=NST, h=HPC))
                attTs = [patt.tile([128, S], BF, tag=f"aT{t}", name=f"attTs{t}")
                         for t in range(2)]

                xt_r = xt.rearrange("(d p) s -> p d s", p=128)

                def load_x(sb):
                    c0 = sb * 512
                    nc.sync.dma_start(xall[:, :, c0:c0 + 512],
                                      xt_r[:, :, c0:c0 + 512])

                peonly = phases == "peonly"

                def qkv(sb):
                    # two psum chains interleaved -> alternating-bank matmuls
                    c0 = sb * 512
                    for w3, dst in ((wqt, qTs), (wkt, kTs)):
                        pss = [psQ.tile([128, 512], DT, tag="mm",
                                        name=f"qk{t}") for t in range(2)]
                        for d in range(ndc):
                            for t in range(2):
                                nc.tensor.matmul(
                                    pss[t][:], w3[:, d, t * 128:(t + 1) * 128],
                                    xall[:, d, c0:c0 + 512],
                                    start=(d == 0), stop=(d == ndc - 1))
                        if peonly:
                            continue
                        for t in range(2):
                            nc.vector.tensor_copy(dst[t][:, c0:c0 + 512],
                                                  pss[t][:])
                    for sp in range(2):
                        sts = (sb * 4 + sp * 2, sb * 4 + sp * 2 + 1)
                        pss = [psQ.tile([128, 256], DT, tag="mm",
                                        name=f"v{i}") for i in range(2)]
                        for d in range(ndc):
                            for i, st in enumerate(sts):
                                nc.tensor.matmul(
                                    pss[i][:],
                                    xall[:, d, st * 128:(st + 1) * 128],
                                    wvt[:, d, :],
                                    start=(d == 0), stop=(d == ndc - 1))
                        if peonly:
                            continue
                        for i, st in enumerate(sts):
                            nc.vector.tensor_copy(
                                v_all[:, st, :, 0:64],
                                pss[i][:].rearrange("p (h e) -> p h e", h=HPC))

                def attention(qb, pair):
                    nkv = 4 * qb + 4 if mode == "causal" else NST
                    ncg = nkv // 2
                    q0 = qb * 512
                    accs = [psA.tile([128, 512], DT, tag="acc", name=f"acc{i}")
                            for i in range(2)]
                    if peonly:
                        # matmul stream only: PV reads a const tile, no
                        # exp/kill/normalize -> measures the PE serial floor
                        for cg in range(ncg):
                            for h2 in range(2):
                                scps = psS.tile([128, 2, 512], DT, tag="sc")
                                for j in range(2):
                                    c = 2 * cg + j
                                    off = (max(0, c * 128 - q0)
                                           if mode == "causal" else 0)
                                    nc.tensor.matmul(
                                        scps[:, j, off:512],
                                        kTs[pair][h2 * 64:(h2 + 1) * 64,
                                                  c * 128:(c + 1) * 128],
                                        qTs[pair][h2 * 64:(h2 + 1) * 64,
                                                  q0 + off:q0 + 512],
                                        start=True, stop=True)
                            for j in range(2):
                                c = 2 * cg + j
                                for h2 in range(2):
                                    nc.tensor.matmul(
                                        accs[h2][:],
                                        v_all[:, c, pair * 2 + h2, :],
                                        cst[:, 4:516],
                                        start=(c == 0), stop=(c == nkv - 1))
                        return
                    LAG = 3
                    exp_q = {}
                    for u in range(ncg + LAG):
                        if u < ncg:
                            cg = u
                            if mode == "general":
                                mt = pmask.tile([128, 2, 512], BF, tag="mt")
                                for j in range(2):
                                    c = 2 * cg + j
                                    nc.sync.dma_start(
                                        mt[:, j, :],
                                        mtin[c * 128:(c + 1) * 128, q0:q0 + 512])
                            for h2 in range(2):
                                lo, hi = h2 * 64, (h2 + 1) * 64
                                scps = psS.tile([128, 2, 512], DT, tag="sc")
                                for j in range(2):
                                    c = 2 * cg + j
                                    off = (max(0, c * 128 - q0)
                                           if mode == "causal" else 0)
                                    nc.tensor.matmul(
                                        scps[:, j, off:512],
                                        kTs[pair][lo:hi, c * 128:(c + 1) * 128],
                                        qTs[pair][lo:hi, q0 + off:q0 + 512],
                                        start=True, stop=True)
                                expt = pexp.tile([128, 2, 512], BF, tag="exp")
                                nc.scalar.activation(expt[:], scps[:], Act.Exp,
                                                     scale=0.125)
                                # causal kill: only the 128-wide diagonal
                                # triangle needs masking -- the fully-masked
                                # rectangle [0:off) is never read (the PV
                                # matmul below skips those columns).
                                # On DVE (bf16 2x); gpsimd is far slower.
                                eng = nc.vector
                                for j in range(2):
                                    c = 2 * cg + j
                                    if mode == "causal" and c * 128 >= q0:
                                        off = c * 128 - q0
                                        eng.tensor_tensor(
                                            expt[:, j, off:off + 128],
                                            expt[:, j, off:off + 128],
                                            cst[:, 388:516], MUL)
                                    elif mode == "general":
                                        eng.tensor_tensor(
                                            expt[:, j, :], expt[:, j, :],
                                            mt[:, j, :], MUL)
                                exp_q[(cg, h2)] = expt
                        if u >= LAG:
                            cg = u - LAG
                            es = [exp_q.pop((cg, 0)), exp_q.pop((cg, 1))]
                            # j-outer so consecutive matmuls alternate the
                            # two accumulator banks; diagonal chunks skip
                            # the fully-masked columns [0:off) entirely
                            # (element-wise has_written keeps them intact)
                            for j in range(2):
                                c = 2 * cg + j
                                off = (max(0, c * 128 - q0)
                                       if mode == "causal" else 0)
                                for h2 in range(2):
                                    nc.tensor.matmul(
                                        accs[h2][:, off:512],
                                        v_all[:, c, pair * 2 + h2, :],
                                        es[h2][:, j, off:512],
                                        start=(c == 0), stop=(c == nkv - 1))
                    for h2 in range(2):
                        # accs rows 64:128 hold the denominator (replicated
                        # by the ones columns of v'); normalize on DVE only
                        if phases == "nonorm":
                            nc.vector.tensor_copy(
                                attTs[pair][h2 * 64:(h2 + 1) * 64,
                                            q0:q0 + 512],
                                accs[h2][0:64, :])
                            continue
                        if phases == "denom":
                            nc.vector.tensor_copy(
                                attTs[pair][h2 * 64:(h2 + 1) * 64,
                                            q0:q0 + 512],
                                accs[h2][64:128, :])
                            continue
                        recipb = pmisc.tile([64, 512], DT, tag="recipb")
                        nc.vector.reciprocal(recipb[:], accs[h2][64:128, :])
                        nc.vector.tensor_tensor(
                            attTs[pair][h2 * 64:(h2 + 1) * 64, q0:q0 + 512],
                            accs[h2][0:64, :], recipb[:], MUL)

                def proj(qb):
                    # t-outer so the attT stationary is loaded once per two
                    # matmuls (two psum chains, one per output half)
                    for stl in range(4):
                        st = qb * 4 + stl
                        ot = pout.tile([128, D], BF, tag="out")
                        pss = [psQ.tile([128, 512], DT, tag="mm",
                                        name=f"pj{half}") for half in range(2)]
                        for t in range(2):
                            for half in range(2):
                                nc.tensor.matmul(
                                    pss[half][:],
                                    attTs[t][:, st * 128:(st + 1) * 128],
                                    wot[:, t, half * 512:(half + 1) * 512],
                                    start=(t == 0), stop=(t == 1))
                        if peonly:
                            continue
                        for half in range(2):
                            nc.vector.tensor_copy(
                                ot[:, half * 512:(half + 1) * 512],
                                pss[half][:])
                        nc.sync.dma_start(outp[st * 128:(st + 1) * 128, :],
                                          ot[:])

                load_x(0)
                nc.sync.dma_start(wvt[:], wv.rearrange("(d p) j -> p d j", p=128))
                nc.sync.dma_start(wot[:], wo.rearrange("(t p) j -> p t j", p=128))
                nc.sync.dma_start(cst[:], cin[:, 0:516])
                if mode == "causal":
                    # att(qb) only reads kv chunks <= qb: issue it as early
                    # as its deps allow so the ACT exp stream starts ~10us
                    # in; qkv(sb+1)/proj become the PE filler that the
                    # scheduler pulls forward whenever attention stalls on
                    # exp results.
                    qkv(0)
                    load_x(1)
                    attention(0, 0); attention(0, 1)
                    qkv(1)
                    load_x(2)
                    attention(1, 0); attention(1, 1)
                    qkv(2)
                    load_x(3)
                    attention(2, 0); attention(2, 1)
                    qkv(3)
                    attention(3, 0)
                    proj(0); proj(1)
                    attention(3, 1)
                    proj(2); proj(3)
                else:
                    # full attention reads every kv chunk: all qkv first
                    qkv(0)
                    load_x(1); qkv(1)
                    load_x(2); qkv(2)
                    load_x(3); qkv(3)
                    for qb in range(NSB):
                        attention(qb, 0); attention(qb, 1)
                        proj(qb)

            if reps == 1:
                body()
            else:
                with tc.For_i(0, reps, 1):
                    body()

    nc.compile()
    return nc


def _consts_array():
    c = np.zeros((128, 4612), dtype=np.float32)
    c[:, 0:4] = 1.0
    c[:, 388:516] = np.triu(np.ones((128, 128), np.float32))
    c[:, 516:4612] = 1.0
    return c


def _bf16(a):
    import ml_dtypes
    return np.ascontiguousarray(np.asarray(a, np.float32)).astype(
        ml_dtypes.bfloat16)


def make_in_maps(x, mask, Wq, bq, Wk, bk, Wv, bv, Wo, bo):
    x = np.asarray(x, np.float32)
    m = np.asarray(mask)[0, 0]
    mb = (m != 0)
    if mb.all():
        mode = "none"
    elif np.array_equal(mb, np.tril(np.ones((S, S), bool))):
        mode = "causal"
    else:
        mode = "general"
    has_bias = bool(np.any(bq) or np.any(bk) or np.any(bv))

    Wq = np.asarray(Wq, np.float32)
    Wk = np.asarray(Wk, np.float32)
    Wv = np.asarray(Wv, np.float32)
    Wo = np.asarray(Wo, np.float32)
    if has_bias:
        # contraction dim padded to 9*128: row D = bias (ones row in xT)
        pad = np.zeros((128, H * DH), np.float32)
        Wq = np.concatenate([Wq, pad], 0)
        Wk = np.concatenate([Wk, pad], 0)
        Wv = np.concatenate([Wv, pad], 0)
        Wq[D] = np.asarray(bq, np.float32)
        Wk[D] = np.asarray(bk, np.float32)
        Wv[D] = np.asarray(bv, np.float32)
    consts = _consts_array()
    maskt = mb.T.astype(np.float32) if mode == "general" else None

    in_maps = []
    for c in range(NCORES):
        b, hg = divmod(c, HPC)
        cols = slice(hg * DIMS, (hg + 1) * DIMS)
        xtb = x[b].T
        if has_bias:
            xtb = np.concatenate(
                [xtb, np.ones((1, S), np.float32),
                 np.zeros((127, S), np.float32)], 0)
        im = {
            "xt": _bf16(xtb),
            "wq": _bf16(Wq[:, cols]),
            "wk": _bf16(Wk[:, cols]),
            "wv": _bf16(Wv[:, cols]),
            "wo": _bf16(Wo[hg * DIMS:(hg + 1) * DIMS, :]),
            "cin": _bf16(consts),
        }
        if maskt is not None:
            im["maskt"] = _bf16(maskt)
        in_maps.append(im)
    return in_maps, mode, has_bias


def gather_output(results, bo):
    out = np.zeros((B, S, D), dtype=np.float32)
    for c in range(NCORES):
        out[c // HPC] += np.asarray(results[c]["outp"], dtype=np.float32)
    out += np.asarray(bo, np.float32)[None, None, :]
    return out


def get_program(mode, has_bias, reps=1, phases=None):
    key = (mode, has_bias, reps, phases)
    if key not in _PROG_CACHE:
        _PROG_CACHE[key] = build_program(mode, has_bias, reps, phases)
    return _PROG_CACHE[key]


def run(in_maps, mode, has_bias, reps=1, phases=None, **kwargs):
    from concourse.bass_utils import run_bass_kernel_spmd
    nc = get_program(mode, has_bias, reps, phases)
    return run_bass_kernel_spmd(nc, in_maps, core_ids=list(range(NCORES)),
                                **kwargs)


def kernel(x, mask, Wq, bq, Wk, bk, Wv, bv, Wo, bo):
    in_maps, mode, has_bias = make_in_maps(x, mask, Wq, bq, Wk, bk, Wv, bv,
                                           Wo, bo)
    r = run(in_maps, mode, has_bias, reps=1)
    return gather_output(r.results, bo)


# revision 46
# speedup vs baseline: 1.1844x; 1.0282x over previous
"""Multi-head attention Trainium2 Bass kernel (v2, bf16).

Problem: B=2, S=2048, D=1024, H=16 heads, DH=64, causal (or arbitrary) mask.
Sharding: 8 cores = data-parallel over B (2) x tensor-parallel over head
groups (4 groups of 4 heads). Each core computes QKV projections for its
head group, attention for its 4 heads, and a partial output projection
(attended @ Wo-shard). Host sums the 4 partials per batch and adds bo.

v2 vs v1:
  - all matmul operands bf16 (f32 PSUM); exp output bf16; output bf16
    (host upcasts + reduces)
  - x is pre-transposed on the host -> no PE transposes, no DVE xT copies
  - program order interleaves qkv(sb+1) and proj(qb-1) with the ACT-bound
    attention(qb) so the PE always has filler work
  - PSUM budget: scores 2x[128,2,512] (4 banks) + accs 2x[65,512] (2) +
    qkv/proj 2x[128,512] (2) = 8 banks

Core-local design ("transposed scores" formulation):
  xT   [1024, S]    bf16, host-pretransposed, DMA'd per (d, sb) chunk
  qT,kT [256, S]    W.T @ xT  (2 tiles of [128, S] per, one per head-pair)
  v'   [S, 4, 65]   v natural + ones column per head (PV matmul emits the
                    softmax denominator as row 64)
  sT   [kv, q]      scores transposed = kT-chunk.T @ qT; K=64 matmuls
                    row-tiled (two heads on the two 64-row PE tile rows)
  expT exp(s/8)     bf16; causality: kv-blocks > q skipped, ranges
                    restricted, upper-tri 0/1 template on diagonal blocks
  attT [65, q]      v'.T @ expT accumulated over kv chunks; row 64 =
                    denom; normalize via reciprocal + partition_broadcast
                    + multiply -> attT [128, S] bf16
  out  [S, 1024]    attT.T @ Wo-shard bf16 (partial; host reduces)
"""
import numpy as np

B, S, D = 2, 2048, 1024
H, DH = 16, 64
NCORES = 8
HPC = 4              # heads per core
DIMS = HPC * DH      # 256 projection dims per core
NSB = S // 512       # 4 q/s blocks of 512
NST = S // 128       # 16 s tiles of 128
NDC = D // 128       # 8 contraction chunks

_PROG_CACHE = {}

# build-time tuning knobs (A/B tested; defaults = shipping config)
KNOBS = {"qk_first": False, "exp_split": True, "lag": 3}


def build_program(mode="causal", has_bias=False, reps=1, phases=None):
    import concourse.bacc as bacc
    import concourse.mybir as mybir
    import concourse.tile as tile

    DT = mybir.dt.float32
    BF = mybir.dt.bfloat16
    Act = mybir.ActivationFunctionType
    MUL = mybir.AluOpType.mult

    nc = bacc.Bacc("TRN2", target_bir_lowering=False, debug=False,
                   num_devices=NCORES)

    ndc = NDC + 1 if has_bias else NDC
    d_in = ndc * 128
    xt = nc.dram_tensor("xt", [d_in, S], BF, kind="ExternalInput")
    wq = nc.dram_tensor("wq", [d_in, DIMS], BF, kind="ExternalInput")
    wk = nc.dram_tensor("wk", [d_in, DIMS], BF, kind="ExternalInput")
    wv = nc.dram_tensor("wv", [d_in, DIMS], BF, kind="ExternalInput")
    wo = nc.dram_tensor("wo", [DIMS, D], BF, kind="ExternalInput")
    # consts: [:,0:4] ones | [:,4:388] zeros | [:,388:516] upper-tri ones
    # | [:,516:4612] ones (one-shot DMA into v_all's denominator columns)
    cin = nc.dram_tensor("cin", [128, 4612], BF, kind="ExternalInput")
    if mode == "general":
        mtin = nc.dram_tensor("maskt", [S, S], BF, kind="ExternalInput")
    outp = nc.dram_tensor("outp", [S, D], BF, kind="ExternalOutput")

    with tile.TileContext(nc) as tc:
        with (
            tc.tile_pool(name="pw", bufs=1) as pw,
            tc.tile_pool(name="pxa", bufs=1) as pxa,
            tc.tile_pool(name="pqk", bufs=1) as pqk,
            tc.tile_pool(name="pv", bufs=1) as pv,
            tc.tile_pool(name="patt", bufs=1) as patt,
            tc.tile_pool(name="pexp", bufs=10) as pexp,
            tc.tile_pool(name="pmask", bufs=4) as pmask,
            tc.tile_pool(name="pmisc", bufs=4) as pmisc,
            tc.tile_pool(name="pout", bufs=4) as pout,
            tc.tile_pool(name="psS", bufs=2, space="PSUM") as psS,
            tc.tile_pool(name="psA", bufs=2, space="PSUM") as psA,
            tc.tile_pool(name="psQ", bufs=2, space="PSUM") as psQ,
        ):
            def body():
                # zero the score-psum slots: exp reads full banks while the
                # causal matmuls write only [off:512), so first use would
                # otherwise read uninitialized PSUM (exp -> inf -> NaN)
                for _ in range(2):
                    pini = psS.tile([128, 2, 512], DT, tag="sc")
                    nc.vector.memset(pini[:], 0.0)
                # ---- weights + consts ----
                wqt = pw.tile([128, ndc, DIMS], BF, tag="wqt")
                wkt = pw.tile([128, ndc, DIMS], BF, tag="wkt")
                wvt = pw.tile([128, ndc, DIMS], BF, tag="wvt")
                wot = pw.tile([128, 2, D], BF, tag="wot")
                cst = pw.tile([128, 516], BF, tag="cst")
                # qk weights + x(0) first so the first matmuls start ASAP
                nc.sync.dma_start(wqt[:], wq.rearrange("(d p) j -> p d j", p=128))
                nc.sync.dma_start(wkt[:], wk.rearrange("(d p) j -> p d j", p=128))

                xall = pxa.tile([128, ndc, S], BF, tag="xall")
                qTs = [pqk.tile([128, S], BF, tag=f"qT{t}", name=f"qTs{t}")
                       for t in range(2)]
                kTs = [pqk.tile([128, S], BF, tag=f"kT{t}", name=f"kTs{t}")
                       for t in range(2)]
                # v' = [v | 64 ones cols]: the PV matmul then emits the
                # softmax denominator pre-replicated on partitions 64:128,
                # so no gpsimd partition_broadcast is needed.
                v_all = pv.tile([128, NST, HPC, 128], BF, tag="v")
                nc.sync.dma_start(
                    v_all[:, :, :, 64:128],
                    cin[:, 516:4612].rearrange("p (s h e) -> p s h e",
                                               s=NST, h=HPC))
                attTs = [patt.tile([128, S], BF, tag=f"aT{t}", name=f"attTs{t}")
                         for t in range(2)]

                xt_r = xt.rearrange("(d p) s -> p d s", p=128)

                def load_x(sb):
                    c0 = sb * 512
                    nc.sync.dma_start(xall[:, :, c0:c0 + 512],
                                      xt_r[:, :, c0:c0 + 512])

                peonly = phases == "peonly"

                def qkv(sb):
                    # two psum chains interleaved -> alternating-bank matmuls.
                    # qk_first: pair the q/k chains of the SAME t so pair-t's
                    # attention (scores -> exp, the ACT pacer) can start as
                    # soon as its own projections land.
                    c0 = sb * 512
                    if KNOBS["qk_first"]:
                        groups = [((wqt, qTs, t), (wkt, kTs, t))
                                  for t in range(2)]
                    else:
                        groups = [((wqt, qTs, 0), (wqt, qTs, 1)),
                                  ((wkt, kTs, 0), (wkt, kTs, 1))]
                    for grp in groups:
                        pss = [psQ.tile([128, 512], DT, tag="mm",
                                        name=f"qk{i}") for i in range(2)]
                        for d in range(ndc):
                            for i, (w3, dst, t) in enumerate(grp):
                                nc.tensor.matmul(
                                    pss[i][:], w3[:, d, t * 128:(t + 1) * 128],
                                    xall[:, d, c0:c0 + 512],
                                    start=(d == 0), stop=(d == ndc - 1))
                        if peonly:
                            continue
                        for i, (w3, dst, t) in enumerate(grp):
                            nc.vector.tensor_copy(dst[t][:, c0:c0 + 512],
                                                  pss[i][:])
                    for sp in range(2):
                        sts = (sb * 4 + sp * 2, sb * 4 + sp * 2 + 1)
                        pss = [psQ.tile([128, 256], DT, tag="mm",
                                        name=f"v{i}") for i in range(2)]
                        for d in range(ndc):
                            for i, st in enumerate(sts):
                                nc.tensor.matmul(
                                    pss[i][:],
                                    xall[:, d, st * 128:(st + 1) * 128],
                                    wvt[:, d, :],
                                    start=(d == 0), stop=(d == ndc - 1))
                        if peonly:
                            continue
                        for i, st in enumerate(sts):
                            nc.vector.tensor_copy(
                                v_all[:, st, :, 0:64],
                                pss[i][:].rearrange("p (h e) -> p h e", h=HPC))

                def attention(qb, pair):
                    nkv = 4 * qb + 4 if mode == "causal" else NST
                    ncg = nkv // 2
                    q0 = qb * 512
                    accs = [psA.tile([128, 512], DT, tag="acc", name=f"acc{i}")
                            for i in range(2)]
                    if peonly:
                        # matmul stream only: PV reads a const tile, no
                        # exp/kill/normalize -> measures the PE serial floor
                        for cg in range(ncg):
                            for h2 in range(2):
                                scps = psS.tile([128, 2, 512], DT, tag="sc")
                                for j in range(2):
                                    c = 2 * cg + j
                                    off = (max(0, c * 128 - q0)
                                           if mode == "causal" else 0)
                                    nc.tensor.matmul(
                                        scps[:, j, off:512],
                                        kTs[pair][h2 * 64:(h2 + 1) * 64,
                                                  c * 128:(c + 1) * 128],
                                        qTs[pair][h2 * 64:(h2 + 1) * 64,
                                                  q0 + off:q0 + 512],
                                        start=True, stop=True)
                            for j in range(2):
                                c = 2 * cg + j
                                for h2 in range(2):
                                    nc.tensor.matmul(
                                        accs[h2][:],
                                        v_all[:, c, pair * 2 + h2, :],
                                        cst[:, 4:516],
                                        start=(c == 0), stop=(c == nkv - 1))
                        return
                    LAG = KNOBS["lag"]
                    exp_q = {}
                    for u in range(ncg + LAG):
                        if u < ncg:
                            cg = u
                            if mode == "general":
                                mt = pmask.tile([128, 2, 512], BF, tag="mt")
                                for j in range(2):
                                    c = 2 * cg + j
                                    nc.sync.dma_start(
                                        mt[:, j, :],
                                        mtin[c * 128:(c + 1) * 128, q0:q0 + 512])
                            for h2 in range(2):
                                lo, hi = h2 * 64, (h2 + 1) * 64
                                scps = psS.tile([128, 2, 512], DT, tag="sc")
                                for j in range(2):
                                    c = 2 * cg + j
                                    off = (max(0, c * 128 - q0)
                                           if mode == "causal" else 0)
                                    nc.tensor.matmul(
                                        scps[:, j, off:512],
                                        kTs[pair][lo:hi, c * 128:(c + 1) * 128],
                                        qTs[pair][lo:hi, q0 + off:q0 + 512],
                                        start=True, stop=True)
                                expt = pexp.tile([128, 2, 512], BF, tag="exp")
                                off0 = (max(0, 2 * cg * 128 - q0)
                                        if mode == "causal" else 0)
                                if KNOBS["exp_split"] and off0 >= 256:
                                    # deep-diagonal: cols below off are never
                                    # read; ranged activations save ACT time
                                    for j in range(2):
                                        offj = (2 * cg + j) * 128 - q0
                                        nc.scalar.activation(
                                            expt[:, j, offj:512],
                                            scps[:, j, offj:512],
                                            Act.Exp, scale=0.125)
                                else:
                                    nc.scalar.activation(expt[:], scps[:],
                                                         Act.Exp, scale=0.125)
                                # causal kill: only the 128-wide diagonal
                                # triangle needs masking -- the fully-masked
                                # rectangle [0:off) is never read (the PV
                                # matmul below skips those columns).
                                # On DVE (bf16 2x); gpsimd is far slower.
                                eng = nc.vector
                                for j in range(2):
                                    c = 2 * cg + j
                                    if mode == "causal" and c * 128 >= q0:
                                        off = c * 128 - q0
                                        eng.tensor_tensor(
                                            expt[:, j, off:off + 128],
                                            expt[:, j, off:off + 128],
                                            cst[:, 388:516], MUL)
                                    elif mode == "general":
                                        eng.tensor_tensor(
                                            expt[:, j, :], expt[:, j, :],
                                            mt[:, j, :], MUL)
                                exp_q[(cg, h2)] = expt
                        if u >= LAG:
                            cg = u - LAG
                            es = [exp_q.pop((cg, 0)), exp_q.pop((cg, 1))]
                            # j-outer so consecutive matmuls alternate the
                            # two accumulator banks; diagonal chunks skip
                            # the fully-masked columns [0:off) entirely
                            # (element-wise has_written keeps them intact)
                            for j in range(2):
                                c = 2 * cg + j
                                off = (max(0, c * 128 - q0)
                                       if mode == "causal" else 0)
                                for h2 in range(2):
                                    nc.tensor.matmul(
                                        accs[h2][:, off:512],
                                        v_all[:, c, pair * 2 + h2, :],
                                        es[h2][:, j, off:512],
                                        start=(c == 0), stop=(c == nkv - 1))
                    for h2 in range(2):
                        # accs rows 64:128 hold the denominator (replicated
                        # by the ones columns of v'); normalize on DVE only
                        if phases == "nonorm":
                            nc.vector.tensor_copy(
                                attTs[pair][h2 * 64:(h2 + 1) * 64,
                                            q0:q0 + 512],
                                accs[h2][0:64, :])
                            continue
                        if phases == "denom":
                            nc.vector.tensor_copy(
                                attTs[pair][h2 * 64:(h2 + 1) * 64,
                                            q0:q0 + 512],
                                accs[h2][64:128, :])
                            continue
                        recipb = pmisc.tile([64, 512], DT, tag="recipb")
                        nc.vector.reciprocal(recipb[:], accs[h2][64:128, :])
                        nc.vector.tensor_tensor(
                            attTs[pair][h2 * 64:(h2 + 1) * 64, q0:q0 + 512],
                            accs[h2][0:64, :], recipb[:], MUL)

                def proj(qb):
                    # t-outer so the attT stationary is loaded once per two
                    # matmuls (two psum chains, one per output half)
                    for stl in range(4):
                        st = qb * 4 + stl
                        ot = pout.tile([128, D], BF, tag="out")
                        pss = [psQ.tile([128, 512], DT, tag="mm",
                                        name=f"pj{half}") for half in range(2)]
                        for t in range(2):
                            for half in range(2):
                                nc.tensor.matmul(
                                    pss[half][:],
                                    attTs[t][:, st * 128:(st + 1) * 128],
                                    wot[:, t, half * 512:(half + 1) * 512],
                                    start=(t == 0), stop=(t == 1))
                        if peonly:
                            continue
                        for half in range(2):
                            nc.vector.tensor_copy(
                                ot[:, half * 512:(half + 1) * 512],
                                pss[half][:])
                        nc.sync.dma_start(outp[st * 128:(st + 1) * 128, :],
                                          ot[:])

                load_x(0)
                nc.sync.dma_start(wvt[:], wv.rearrange("(d p) j -> p d j", p=128))
                nc.sync.dma_start(wot[:], wo.rearrange("(t p) j -> p t j", p=128))
                nc.sync.dma_start(cst[:], cin[:, 0:516])
                if mode == "causal":
                    # att(qb) only reads kv chunks <= qb: issue it as early
                    # as its deps allow so the ACT exp stream starts ~10us
                    # in; qkv(sb+1)/proj become the PE filler that the
                    # scheduler pulls forward whenever attention stalls on
                    # exp results.
                    qkv(0)
                    load_x(1)
                    attention(0, 0); attention(0, 1)
                    qkv(1)
                    load_x(2)
                    attention(1, 0); attention(1, 1)
                    qkv(2)
                    load_x(3)
                    attention(2, 0); attention(2, 1)
                    qkv(3)
                    attention(3, 0)
                    proj(0); proj(1)
                    attention(3, 1)
                    proj(2); proj(3)
                else:
                    # full attention reads every kv chunk: all qkv first
                    qkv(0)
                    load_x(1); qkv(1)
                    load_x(2); qkv(2)
                    load_x(3); qkv(3)
                    for qb in range(NSB):
                        attention(qb, 0); attention(qb, 1)
                        proj(qb)

            if reps == 1:
                body()
            else:
                with tc.For_i(0, reps, 1):
                    body()

    nc.compile()
    return nc


def _consts_array():
    c = np.zeros((128, 4612), dtype=np.float32)
    c[:, 0:4] = 1.0
    c[:, 388:516] = np.triu(np.ones((128, 128), np.float32))
    c[:, 516:4612] = 1.0
    return c


def _bf16(a):
    import ml_dtypes
    return np.ascontiguousarray(np.asarray(a, np.float32)).astype(
        ml_dtypes.bfloat16)


def make_in_maps(x, mask, Wq, bq, Wk, bk, Wv, bv, Wo, bo):
    x = np.asarray(x, np.float32)
    m = np.asarray(mask)[0, 0]
    mb = (m != 0)
    if mb.all():
        mode = "none"
    elif np.array_equal(mb, np.tril(np.ones((S, S), bool))):
        mode = "causal"
    else:
        mode = "general"
    has_bias = bool(np.any(bq) or np.any(bk) or np.any(bv))

    Wq = np.asarray(Wq, np.float32)
    Wk = np.asarray(Wk, np.float32)
    Wv = np.asarray(Wv, np.float32)
    Wo = np.asarray(Wo, np.float32)
    if has_bias:
        # contraction dim padded to 9*128: row D = bias (ones row in xT)
        pad = np.zeros((128, H * DH), np.float32)
        Wq = np.concatenate([Wq, pad], 0)
        Wk = np.concatenate([Wk, pad], 0)
        Wv = np.concatenate([Wv, pad], 0)
        Wq[D] = np.asarray(bq, np.float32)
        Wk[D] = np.asarray(bk, np.float32)
        Wv[D] = np.asarray(bv, np.float32)
    consts = _consts_array()
    maskt = mb.T.astype(np.float32) if mode == "general" else None

    in_maps = []
    for c in range(NCORES):
        b, hg = divmod(c, HPC)
        cols = slice(hg * DIMS, (hg + 1) * DIMS)
        xtb = x[b].T
        if has_bias:
            xtb = np.concatenate(
                [xtb, np.ones((1, S), np.float32),
                 np.zeros((127, S), np.float32)], 0)
        im = {
            "xt": _bf16(xtb),
            "wq": _bf16(Wq[:, cols]),
            "wk": _bf16(Wk[:, cols]),
            "wv": _bf16(Wv[:, cols]),
            "wo": _bf16(Wo[hg * DIMS:(hg + 1) * DIMS, :]),
            "cin": _bf16(consts),
        }
        if maskt is not None:
            im["maskt"] = _bf16(maskt)
        in_maps.append(im)
    return in_maps, mode, has_bias


def gather_output(results, bo):
    out = np.zeros((B, S, D), dtype=np.float32)
    for c in range(NCORES):
        out[c // HPC] += np.asarray(results[c]["outp"], dtype=np.float32)
    out += np.asarray(bo, np.float32)[None, None, :]
    return out


def get_program(mode, has_bias, reps=1, phases=None):
    key = (mode, has_bias, reps, phases, tuple(sorted(KNOBS.items())))
    if key not in _PROG_CACHE:
        _PROG_CACHE[key] = build_program(mode, has_bias, reps, phases)
    return _PROG_CACHE[key]


def run(in_maps, mode, has_bias, reps=1, phases=None, **kwargs):
    from concourse.bass_utils import run_bass_kernel_spmd
    nc = get_program(mode, has_bias, reps, phases)
    return run_bass_kernel_spmd(nc, in_maps, core_ids=list(range(NCORES)),
                                **kwargs)


def kernel(x, mask, Wq, bq, Wk, bk, Wv, bv, Wo, bo):
    in_maps, mode, has_bias = make_in_maps(x, mask, Wq, bq, Wk, bk, Wv, bv,
                                           Wo, bo)
    r = run(in_maps, mode, has_bias, reps=1)
    return gather_output(r.results, bo)
